# revision 2
# baseline (speedup 1.0000x reference)
"""BatchedMoE Trainium2 kernel.

Expert-parallel over 8 NeuronCores: host computes the (tiny) router +
top-2 dispatch in numpy; core c runs expert c's gated MLP over the
tokens routed to it (capacity-padded), plus the shared-expert MLP for
a 1/8 slice of all tokens. All heavy matmuls run on-device in fp32r
(full-rate fp32). Host scatters/combines the partial outputs.

Self-contained: only numpy + concourse imports, no sibling files.
"""
import numpy as np

B, T, C = 4, 2048, 1024
E = 8            # experts == cores
KTOP = 2         # experts per token
H = 1408         # expert intermediate
HS = 2816        # shared intermediate
N = B * T        # 8192 tokens
TPC = N // 8     # tokens per core for the shared expert
KC = C // 128    # 8 k-tiles over C
NH = H // 128    # 11 h-tiles
NHS = HS // 128  # 22 hs-tiles
NC2 = C // 512   # 2 c-halves

TRACE = False
LAST_EXEC_NS = None
LAST_RESULTS = None

_cache = {}


def _build(cap):
    import concourse.bacc as bacc
    import concourse.tile as tile
    import concourse.mybir as mybir
    from contextlib import ExitStack

    f32 = mybir.dt.float32
    f32r = mybir.dt.float32r
    AF = mybir.ActivationFunctionType

    nc = bacc.Bacc("TRN2", target_bir_lowering=False, debug=False)

    xdT = nc.dram_tensor("xdT", [C, cap], f32, kind="ExternalInput").ap()
    w1 = nc.dram_tensor("w1", [C, H], f32, kind="ExternalInput").ap()
    w2 = nc.dram_tensor("w2", [C, H], f32, kind="ExternalInput").ap()
    w3 = nc.dram_tensor("w3", [H, C], f32, kind="ExternalInput").ap()
    probs = nc.dram_tensor("probs", [128, cap], f32, kind="ExternalInput").ap()
    xsT = nc.dram_tensor("xsT", [C, TPC], f32, kind="ExternalInput").ap()
    ws1b = nc.dram_tensor("ws1b", [NHS, 128, C], f32, kind="ExternalInput").ap()
    ws2b = nc.dram_tensor("ws2b", [NHS, 128, C], f32, kind="ExternalInput").ap()
    ws3 = nc.dram_tensor("ws3", [HS, C], f32, kind="ExternalInput").ap()
    yd = nc.dram_tensor("yd", [cap, C], f32, kind="ExternalOutput").ap()
    ys = nc.dram_tensor("ys", [TPC, C], f32, kind="ExternalOutput").ap()

    groups = []
    s = 0
    while s < cap:
        w = min(512, cap - s)
        groups.append((s, w))
        s += w

    with tile.TileContext(nc) as tc:
        # ---------------- Phase A: routed expert ----------------
        with ExitStack() as pa:
            wp = pa.enter_context(tc.tile_pool(name="wA", bufs=1))
            xp = pa.enter_context(tc.tile_pool(name="xA", bufs=1))
            hp = pa.enter_context(tc.tile_pool(name="hA", bufs=1))
            pp = pa.enter_context(tc.tile_pool(name="pbc", bufs=2))
            sp = pa.enter_context(tc.tile_pool(name="tmpA", bufs=2))
            op = pa.enter_context(tc.tile_pool(name="outA", bufs=2))
            psh = pa.enter_context(tc.tile_pool(name="psA", bufs=4, space="PSUM"))
            psy = pa.enter_context(tc.tile_pool(name="psyA", bufs=2, space="PSUM"))

            # First group's tokens before the big weight slabs so the PE can
            # start as soon as the first k-slab lands.
            g0s, g0w = groups[0]
            xg = [xp.tile([128, g0w], f32r, tag=f"x{k}", name=f"xg{k}") for k in range(KC)]
            for k in range(KC):
                nc.sync.dma_start(
                    xg[k][:], xdT[k * 128:(k + 1) * 128, g0s:g0s + g0w].bitcast(f32r))
            pb = pp.tile([128, g0w], f32, tag="pb")
            nc.sync.dma_start(pb[:], probs[:, g0s:g0s + g0w])

            w1sb, w2sb = [], []
            for k in range(KC):
                t1 = wp.tile([128, H], f32r, tag=f"w1_{k}")
                nc.sync.dma_start(t1[:], w1[k * 128:(k + 1) * 128, :].bitcast(f32r))
                w1sb.append(t1)
                t2 = wp.tile([128, H], f32r, tag=f"w2_{k}")
                nc.sync.dma_start(t2[:], w2[k * 128:(k + 1) * 128, :].bitcast(f32r))
                w2sb.append(t2)
            w3sb = []
            for h in range(NH):
                t3 = wp.tile([128, C], f32r, tag=f"w3_{h}")
                nc.sync.dma_start(t3[:], w3[h * 128:(h + 1) * 128, :].bitcast(f32r))
                w3sb.append(t3)

            for gi, (gs, gw) in enumerate(groups):
                if gi > 0:
                    xg = [xp.tile([128, gw], f32r, tag=f"x{k}", name=f"xg{k}") for k in range(KC)]
                    for k in range(KC):
                        nc.sync.dma_start(
                            xg[k][:],
                            xdT[k * 128:(k + 1) * 128, gs:gs + gw].bitcast(f32r))
                    pb = pp.tile([128, gw], f32, tag="pb")
                    nc.sync.dma_start(pb[:], probs[:, gs:gs + gw])

                hts = []
                for h in range(NH):
                    p1 = psh.tile([128, gw], f32, tag="ph")
                    for k in range(KC):
                        nc.tensor.matmul(
                            p1[:], w1sb[k][:, h * 128:(h + 1) * 128], xg[k][:],
                            start=(k == 0), stop=(k == KC - 1))
                    p2 = psh.tile([128, gw], f32, tag="ph")
                    for k in range(KC):
                        nc.tensor.matmul(
                            p2[:], w2sb[k][:, h * 128:(h + 1) * 128], xg[k][:],
                            start=(k == 0), stop=(k == KC - 1))
                    sl = sp.tile([128, gw], f32, tag="sl")
                    nc.scalar.activation(sl[:], p1[:], AF.Silu)
                    t2 = sp.tile([128, gw], f32, tag="t2")
                    nc.vector.tensor_mul(t2[:], p2[:], pb[:])
                    ht = hp.tile([128, gw], f32r, tag=f"h{h}")
                    nc.vector.tensor_mul(ht[:], sl[:], t2[:])
                    hts.append(ht)

                for t in range(gw // 128):
                    for c in range(NC2):
                        py = psy.tile([128, 512], f32, tag="py")
                        for h in range(NH):
                            nc.tensor.matmul(
                                py[:], hts[h][:, t * 128:(t + 1) * 128],
                                w3sb[h][:, c * 512:(c + 1) * 512],
                                start=(h == 0), stop=(h == NH - 1))
                        ot = op.tile([128, 512], f32, tag="ot")
                        nc.vector.tensor_copy(ot[:], py[:])
                        nc.sync.dma_start(
                            yd[gs + t * 128: gs + (t + 1) * 128,
                               c * 512:(c + 1) * 512], ot[:])

        # ---------------- Phase B: shared expert ----------------
        with tc.tile_pool(name="hsB", bufs=1) as hbp:
            hst = [hbp.tile([128, TPC], f32r, tag=f"hs{j}", name=f"hst{j}") for j in range(NHS)]
            with ExitStack() as pb1:
                xsp = pb1.enter_context(tc.tile_pool(name="xsB", bufs=1))
                cbp = pb1.enter_context(tc.tile_pool(name="cbB", bufs=2))
                spB = pb1.enter_context(tc.tile_pool(name="tmpB", bufs=2))
                psB = pb1.enter_context(tc.tile_pool(name="psB", bufs=4, space="PSUM"))

                xsb = [xsp.tile([128, TPC], f32r, tag=f"xs{k}", name=f"xsb{k}") for k in range(KC)]
                for k in range(KC):
                    nc.sync.dma_start(
                        xsb[k][:], xsT[k * 128:(k + 1) * 128, :].bitcast(f32r))

                for j in range(NHS):
                    cb1 = cbp.tile([128, C], f32r, tag="cb1")
                    nc.sync.dma_start(cb1[:], ws1b[j, :, :].bitcast(f32r))
                    cb2 = cbp.tile([128, C], f32r, tag="cb2")
                    nc.sync.dma_start(cb2[:], ws2b[j, :, :].bitcast(f32r))
                    for th in range(TPC // 512):
                        p1 = psB.tile([128, 512], f32, tag="pB")
                        for k in range(KC):
                            nc.tensor.matmul(
                                p1[:], cb1[:, k * 128:(k + 1) * 128],
                                xsb[k][:, th * 512:(th + 1) * 512],
                                start=(k == 0), stop=(k == KC - 1))
                        p2 = psB.tile([128, 512], f32, tag="pB")
                        for k in range(KC):
                            nc.tensor.matmul(
                                p2[:], cb2[:, k * 128:(k + 1) * 128],
                                xsb[k][:, th * 512:(th + 1) * 512],
                                start=(k == 0), stop=(k == KC - 1))
                        sl = spB.tile([128, 512], f32, tag="slB")
                        nc.scalar.activation(sl[:], p1[:], AF.Silu)
                        nc.vector.tensor_mul(
                            hst[j][:, th * 512:(th + 1) * 512], sl[:], p2[:])

            with ExitStack() as pb2:
                w3p = pb2.enter_context(tc.tile_pool(name="ws3B", bufs=1))
                oB = pb2.enter_context(tc.tile_pool(name="outB", bufs=2))
                psyB = pb2.enter_context(
                    tc.tile_pool(name="psyB", bufs=2, space="PSUM"))
                ws3sb = []
                for j in range(NHS):
                    t3 = w3p.tile([128, C], f32r, tag=f"ws3_{j}")
                    nc.sync.dma_start(
                        t3[:], ws3[j * 128:(j + 1) * 128, :].bitcast(f32r))
                    ws3sb.append(t3)
                for t in range(TPC // 128):
                    for c in range(NC2):
                        py = psyB.tile([128, 512], f32, tag="pyB")
                        for j in range(NHS):
                            nc.tensor.matmul(
                                py[:], hst[j][:, t * 128:(t + 1) * 128],
                                ws3sb[j][:, c * 512:(c + 1) * 512],
                                start=(j == 0), stop=(j == NHS - 1))
                        ot = oB.tile([128, 512], f32, tag="otB")
                        nc.vector.tensor_copy(ot[:], py[:])
                        nc.sync.dma_start(
                            ys[t * 128:(t + 1) * 128, c * 512:(c + 1) * 512], ot[:])

    nc.compile()
    return nc


def _get_nc(cap):
    if cap not in _cache:
        _cache[cap] = _build(cap)
    return _cache[cap]


def kernel(x, Wg, W1, W2, W3, Ws1, Ws2, Ws3):
    global LAST_EXEC_NS, LAST_RESULTS
    from concourse import bass_utils

    x = np.ascontiguousarray(np.asarray(x, dtype=np.float32))
    Wg = np.asarray(Wg, dtype=np.float32)
    W1 = np.ascontiguousarray(np.asarray(W1, dtype=np.float32))
    W2 = np.ascontiguousarray(np.asarray(W2, dtype=np.float32))
    W3 = np.ascontiguousarray(np.asarray(W3, dtype=np.float32))
    Ws1 = np.asarray(Ws1, dtype=np.float32)
    Ws2 = np.asarray(Ws2, dtype=np.float32)
    Ws3 = np.ascontiguousarray(np.asarray(Ws3, dtype=np.float32))

    xf = x.reshape(N, C)

    # ---- router + top-2 + softmax (fp32, matches jax.lax.top_k tie-break) ----
    router = xf @ Wg                                   # [N, E]
    i0 = np.argmax(router, axis=1)
    ar = np.arange(N)
    l0 = router[ar, i0]
    r2 = router.copy()
    r2[ar, i0] = -np.inf
    i1 = np.argmax(r2, axis=1)
    l1 = router[ar, i1]
    m = np.maximum(l0, l1)
    e0 = np.exp(l0 - m)
    e1 = np.exp(l1 - m)
    zs = e0 + e1
    p0 = (e0 / zs).astype(np.float32)
    p1 = (e1 / zs).astype(np.float32)

    # ---- dispatch: sort (token, slot) pairs by expert ----
    flat_e = np.concatenate([i0, i1])                  # [2N]
    flat_t = np.concatenate([ar, ar])
    flat_p = np.concatenate([p0, p1])
    order = np.argsort(flat_e, kind="stable")
    counts = np.bincount(flat_e, minlength=E)
    offs = np.zeros(E + 1, dtype=np.int64)
    np.cumsum(counts, out=offs[1:])

    cap = max(2304, int(-(-counts.max() // 256) * 256))

    # slot of each pair inside its expert's buffer
    slot = np.empty(2 * N, dtype=np.int64)
    slot[order] = np.arange(2 * N) - offs[flat_e[order]]
    gslot = flat_e * cap + slot                        # into stacked [E*cap, C]

    # ---- per-core inputs ----
    ws1b = np.ascontiguousarray(
        Ws1.reshape(KC, 128, NHS, 128).transpose(2, 1, 0, 3).reshape(NHS, 128, C))
    ws2b = np.ascontiguousarray(
        Ws2.reshape(KC, 128, NHS, 128).transpose(2, 1, 0, 3).reshape(NHS, 128, C))

    in_maps = []
    for e in range(E):
        sel = order[offs[e]:offs[e + 1]]
        toks = flat_t[sel]
        pr = flat_p[sel]
        xd = np.zeros((cap, C), dtype=np.float32)
        xd[:len(toks)] = xf[toks]
        pbc = np.zeros((cap,), dtype=np.float32)
        pbc[:len(toks)] = pr
        in_maps.append({
            "xdT": np.ascontiguousarray(xd.T),
            "w1": W1[e],
            "w2": W2[e],
            "w3": W3[e],
            "probs": np.ascontiguousarray(np.broadcast_to(pbc, (128, cap))),
            "xsT": np.ascontiguousarray(xf[e * TPC:(e + 1) * TPC].T),
            "ws1b": ws1b,
            "ws2b": ws2b,
            "ws3": Ws3,
        })

    nc = _get_nc(cap)
    res = bass_utils.run_bass_kernel_spmd(
        nc, in_maps, core_ids=list(range(8)), trace=TRACE)
    LAST_EXEC_NS = res.exec_time_ns
    LAST_RESULTS = res

    # ---- combine ----
    YD = np.concatenate([res.results[e]["yd"] for e in range(E)], axis=0)
    y = YD[gslot[:N]] + YD[gslot[N:]]
    y += np.concatenate([res.results[c]["ys"] for c in range(E)], axis=0)
    return y.reshape(B, T, C)


# revision 8
# speedup vs baseline: 1.0806x; 1.0806x over previous
"""BatchedMoE Trainium2 kernel.

Expert-parallel over 8 NeuronCores: host computes the (tiny) router +
top-2 dispatch in numpy; core c runs expert c's gated MLP over the
tokens routed to it (capacity-padded), plus the shared-expert MLP for
a 1/8 slice of all tokens. All heavy matmuls run on-device in fp32r
(full-rate fp32). Host scatters/combines the partial outputs.

Self-contained: only numpy + concourse imports, no sibling files.
"""
import numpy as np

B, T, C = 4, 2048, 1024
E = 8            # experts == cores
KTOP = 2         # experts per token
H = 1408         # expert intermediate
HS = 2816        # shared intermediate
N = B * T        # 8192 tokens
TPC = N // 8     # tokens per core for the shared expert
KC = C // 128    # 8 k-tiles over C
NH = H // 128    # 11 h-tiles
NHS = HS // 128  # 22 hs-tiles
NC2 = C // 512   # 2 c-halves

TRACE = False
LAST_EXEC_NS = None
LAST_RESULTS = None

_cache = {}


def _build(cap):
    import concourse.bacc as bacc
    import concourse.tile as tile
    import concourse.mybir as mybir
    from contextlib import ExitStack

    f32 = mybir.dt.float32
    f32r = mybir.dt.float32r
    bf16 = mybir.dt.bfloat16
    AF = mybir.ActivationFunctionType

    nc = bacc.Bacc("TRN2", target_bir_lowering=False, debug=False)

    xdT = nc.dram_tensor("xdT", [C, cap], f32, kind="ExternalInput").ap()
    w1 = nc.dram_tensor("w1", [C, H], f32, kind="ExternalInput").ap()
    w2 = nc.dram_tensor("w2", [C, H], f32, kind="ExternalInput").ap()
    w3 = nc.dram_tensor("w3", [H, C], f32, kind="ExternalInput").ap()
    probs = nc.dram_tensor("probs", [128, cap], f32, kind="ExternalInput").ap()
    xsT = nc.dram_tensor("xsT", [C, TPC], f32, kind="ExternalInput").ap()
    ws1b = nc.dram_tensor("ws1b", [NHS, 128, C], f32, kind="ExternalInput").ap()
    ws2b = nc.dram_tensor("ws2b", [NHS, 128, C], f32, kind="ExternalInput").ap()
    ws3 = nc.dram_tensor("ws3", [HS, C], bf16, kind="ExternalInput").ap()
    yd = nc.dram_tensor("yd", [cap, C], f32, kind="ExternalOutput").ap()
    ys = nc.dram_tensor("ys", [TPC, C], f32, kind="ExternalOutput").ap()

    groups = []
    s = 0
    while s < cap:
        w = min(512, cap - s)
        groups.append((s, w))
        s += w

    with tile.TileContext(nc) as tc:
        # ---------------- Phase A: routed expert ----------------
        with ExitStack() as pa:
            wp = pa.enter_context(tc.tile_pool(name="wA", bufs=1))
            xp = pa.enter_context(tc.tile_pool(name="xA", bufs=1))
            hp = pa.enter_context(tc.tile_pool(name="hA", bufs=1))
            pp = pa.enter_context(tc.tile_pool(name="pbc", bufs=2))
            sp = pa.enter_context(tc.tile_pool(name="tmpA", bufs=2))
            op = pa.enter_context(tc.tile_pool(name="outA", bufs=2))
            psh = pa.enter_context(tc.tile_pool(name="psA", bufs=4, space="PSUM"))
            psy = pa.enter_context(tc.tile_pool(name="psyA", bufs=2, space="PSUM"))

            # DMA issue order tracks first-use order: the br1 h=0 chain
            # consumes (w1[k], xg[k]) pairs k-ascending, then br2 needs w2,
            # then ph2 needs w3.
            g0s, g0w = groups[0]
            xg = [xp.tile([128, g0w], f32r, tag=f"x{k}", name=f"xg{k}") for k in range(KC)]
            w1sb = [wp.tile([128, H], f32r, tag=f"w1_{k}", name=f"w1sb{k}") for k in range(KC)]
            w2sb = [wp.tile([128, H], f32r, tag=f"w2_{k}", name=f"w2sb{k}") for k in range(KC)]
            w3sb = [wp.tile([128, C], f32r, tag=f"w3_{h}", name=f"w3sb{h}") for h in range(NH)]
            for k in range(KC):
                nc.sync.dma_start(
                    xg[k][:], xdT[k * 128:(k + 1) * 128, g0s:g0s + g0w].bitcast(f32r))
                nc.sync.dma_start(
                    w1sb[k][:], w1[k * 128:(k + 1) * 128, :].bitcast(f32r))
            pb = pp.tile([128, g0w], f32, tag="pb")
            nc.sync.dma_start(pb[:], probs[:, g0s:g0s + g0w])
            for k in range(KC):
                nc.sync.dma_start(
                    w2sb[k][:], w2[k * 128:(k + 1) * 128, :].bitcast(f32r))
            for h in range(NH):
                nc.sync.dma_start(
                    w3sb[h][:], w3[h * 128:(h + 1) * 128, :].bitcast(f32r))

            for gi, (gs, gw) in enumerate(groups):
                if gi > 0:
                    xg = [xp.tile([128, gw], f32r, tag=f"x{k}", name=f"xg{k}") for k in range(KC)]
                    for k in range(KC):
                        nc.sync.dma_start(
                            xg[k][:],
                            xdT[k * 128:(k + 1) * 128, gs:gs + gw].bitcast(f32r))
                    pb = pp.tile([128, gw], f32, tag="pb")
                    nc.sync.dma_start(pb[:], probs[:, gs:gs + gw])

                hts = []
                for h in range(NH):
                    p1 = psh.tile([128, gw], f32, tag="ph")
                    for k in range(KC):
                        nc.tensor.matmul(
                            p1[:], w1sb[k][:, h * 128:(h + 1) * 128], xg[k][:],
                            start=(k == 0), stop=(k == KC - 1))
                    p2 = psh.tile([128, gw], f32, tag="ph")
                    for k in range(KC):
                        nc.tensor.matmul(
                            p2[:], w2sb[k][:, h * 128:(h + 1) * 128], xg[k][:],
                            start=(k == 0), stop=(k == KC - 1))
                    sl = sp.tile([128, gw], f32, tag="sl")
                    nc.scalar.activation(sl[:], p1[:], AF.Silu)
                    t2 = sp.tile([128, gw], f32, tag="t2")
                    nc.vector.tensor_mul(t2[:], p2[:], pb[:])
                    ht = hp.tile([128, gw], f32r, tag=f"h{h}")
                    nc.vector.tensor_mul(ht[:], sl[:], t2[:])
                    hts.append(ht)

                for t in range(gw // 128):
                    for c in range(NC2):
                        py = psy.tile([128, 512], f32, tag="py")
                        for h in range(NH):
                            nc.tensor.matmul(
                                py[:], hts[h][:, t * 128:(t + 1) * 128],
                                w3sb[h][:, c * 512:(c + 1) * 512],
                                start=(h == 0), stop=(h == NH - 1))
                        ot = op.tile([128, 512], f32, tag="ot")
                        nc.vector.tensor_copy(ot[:], py[:])
                        nc.sync.dma_start(
                            yd[gs + t * 128: gs + (t + 1) * 128,
                               c * 512:(c + 1) * 512], ot[:])

        # ---------------- Phase B: shared expert ----------------
        # hst/ws3 are bf16 so hsT, ws3 slabs and the B1 working set all fit
        # in SBUF at once — ws3 streams in during B1 instead of stalling B2.
        with ExitStack() as pbx:
            hbp = pbx.enter_context(tc.tile_pool(name="hsB", bufs=1))
            w3p = pbx.enter_context(tc.tile_pool(name="ws3B", bufs=1))
            xsp = pbx.enter_context(tc.tile_pool(name="xsB", bufs=1))
            cbp = pbx.enter_context(tc.tile_pool(name="cbB", bufs=2))
            spB = pbx.enter_context(tc.tile_pool(name="tmpB", bufs=2))
            oB = pbx.enter_context(tc.tile_pool(name="outB", bufs=2))
            psB = pbx.enter_context(tc.tile_pool(name="psB", bufs=4, space="PSUM"))
            psyB = pbx.enter_context(tc.tile_pool(name="psyB", bufs=2, space="PSUM"))

            hst = [hbp.tile([128, TPC], bf16, tag=f"hs{j}", name=f"hst{j}") for j in range(NHS)]
            ws3sb = [w3p.tile([128, C], bf16, tag=f"ws3_{j}", name=f"ws3sb{j}") for j in range(NHS)]
            xsb = [xsp.tile([128, TPC], f32r, tag=f"xs{k}", name=f"xsb{k}") for k in range(KC)]

            # j=0 weight blocks first so the first chain can start early.
            cb1 = cbp.tile([128, C], f32r, tag="cb1")
            nc.sync.dma_start(cb1[:], ws1b[0, :, :].bitcast(f32r))
            cb2 = cbp.tile([128, C], f32r, tag="cb2")
            nc.sync.dma_start(cb2[:], ws2b[0, :, :].bitcast(f32r))
            for k in range(KC):
                nc.sync.dma_start(
                    xsb[k][:], xsT[k * 128:(k + 1) * 128, :].bitcast(f32r))

            for j in range(NHS):
                if j > 0:
                    cb1 = cbp.tile([128, C], f32r, tag="cb1")
                    nc.sync.dma_start(cb1[:], ws1b[j, :, :].bitcast(f32r))
                    cb2 = cbp.tile([128, C], f32r, tag="cb2")
                    nc.sync.dma_start(cb2[:], ws2b[j, :, :].bitcast(f32r))
                # pace the B2 weight prefetch: one slab per j step
                nc.sync.dma_start(ws3sb[j][:], ws3[j * 128:(j + 1) * 128, :])
                for th in range(TPC // 512):
                    p1 = psB.tile([128, 512], f32, tag="pB")
                    for k in range(KC):
                        nc.tensor.matmul(
                            p1[:], cb1[:, k * 128:(k + 1) * 128],
                            xsb[k][:, th * 512:(th + 1) * 512],
                            start=(k == 0), stop=(k == KC - 1))
                    p2 = psB.tile([128, 512], f32, tag="pB")
                    for k in range(KC):
                        nc.tensor.matmul(
                            p2[:], cb2[:, k * 128:(k + 1) * 128],
                            xsb[k][:, th * 512:(th + 1) * 512],
                            start=(k == 0), stop=(k == KC - 1))
                    sl = spB.tile([128, 512], f32, tag="slB")
                    nc.scalar.activation(sl[:], p1[:], AF.Silu)
                    nc.vector.tensor_mul(
                        hst[j][:, th * 512:(th + 1) * 512], sl[:], p2[:])

            for t in range(TPC // 128):
                for c in range(NC2):
                    py = psyB.tile([128, 512], f32, tag="pyB")
                    for j in range(NHS):
                        nc.tensor.matmul(
                            py[:], hst[j][:, t * 128:(t + 1) * 128],
                            ws3sb[j][:, c * 512:(c + 1) * 512],
                            start=(j == 0), stop=(j == NHS - 1))
                    ot = oB.tile([128, 512], f32, tag="otB")
                    nc.vector.tensor_copy(ot[:], py[:])
                    nc.sync.dma_start(
                        ys[t * 128:(t + 1) * 128, c * 512:(c + 1) * 512], ot[:])

    nc.compile()
    return nc


def _get_nc(cap):
    if cap not in _cache:
        _cache[cap] = _build(cap)
    return _cache[cap]


def kernel(x, Wg, W1, W2, W3, Ws1, Ws2, Ws3):
    global LAST_EXEC_NS, LAST_RESULTS
    from concourse import bass_utils

    x = np.ascontiguousarray(np.asarray(x, dtype=np.float32))
    Wg = np.asarray(Wg, dtype=np.float32)
    W1 = np.ascontiguousarray(np.asarray(W1, dtype=np.float32))
    W2 = np.ascontiguousarray(np.asarray(W2, dtype=np.float32))
    W3 = np.ascontiguousarray(np.asarray(W3, dtype=np.float32))
    Ws1 = np.asarray(Ws1, dtype=np.float32)
    Ws2 = np.asarray(Ws2, dtype=np.float32)
    Ws3 = np.ascontiguousarray(np.asarray(Ws3, dtype=np.float32))

    xf = x.reshape(N, C)

    # ---- router + top-2 + softmax (fp32, matches jax.lax.top_k tie-break) ----
    router = xf @ Wg                                   # [N, E]
    i0 = np.argmax(router, axis=1)
    ar = np.arange(N)
    l0 = router[ar, i0]
    r2 = router.copy()
    r2[ar, i0] = -np.inf
    i1 = np.argmax(r2, axis=1)
    l1 = router[ar, i1]
    m = np.maximum(l0, l1)
    e0 = np.exp(l0 - m)
    e1 = np.exp(l1 - m)
    zs = e0 + e1
    p0 = (e0 / zs).astype(np.float32)
    p1 = (e1 / zs).astype(np.float32)

    # ---- dispatch: sort (token, slot) pairs by expert ----
    flat_e = np.concatenate([i0, i1])                  # [2N]
    flat_t = np.concatenate([ar, ar])
    flat_p = np.concatenate([p0, p1])
    order = np.argsort(flat_e, kind="stable")
    counts = np.bincount(flat_e, minlength=E)
    offs = np.zeros(E + 1, dtype=np.int64)
    np.cumsum(counts, out=offs[1:])

    cap = max(2304, int(-(-counts.max() // 256) * 256))

    # slot of each pair inside its expert's buffer
    slot = np.empty(2 * N, dtype=np.int64)
    slot[order] = np.arange(2 * N) - offs[flat_e[order]]
    gslot = flat_e * cap + slot                        # into stacked [E*cap, C]

    # ---- per-core inputs ----
    ws1b = np.ascontiguousarray(
        Ws1.reshape(KC, 128, NHS, 128).transpose(2, 1, 0, 3).reshape(NHS, 128, C))
    ws2b = np.ascontiguousarray(
        Ws2.reshape(KC, 128, NHS, 128).transpose(2, 1, 0, 3).reshape(NHS, 128, C))
    import ml_dtypes
    ws3_bf16 = Ws3.astype(ml_dtypes.bfloat16)

    in_maps = []
    for e in range(E):
        sel = order[offs[e]:offs[e + 1]]
        toks = flat_t[sel]
        pr = flat_p[sel]
        xd = np.zeros((cap, C), dtype=np.float32)
        xd[:len(toks)] = xf[toks]
        pbc = np.zeros((cap,), dtype=np.float32)
        pbc[:len(toks)] = pr
        in_maps.append({
            "xdT": np.ascontiguousarray(xd.T),
            "w1": W1[e],
            "w2": W2[e],
            "w3": W3[e],
            "probs": np.ascontiguousarray(np.broadcast_to(pbc, (128, cap))),
            "xsT": np.ascontiguousarray(xf[e * TPC:(e + 1) * TPC].T),
            "ws1b": ws1b,
            "ws2b": ws2b,
            "ws3": ws3_bf16,
        })

    nc = _get_nc(cap)
    res = bass_utils.run_bass_kernel_spmd(
        nc, in_maps, core_ids=list(range(8)), trace=TRACE)
    LAST_EXEC_NS = res.exec_time_ns
    LAST_RESULTS = res

    # ---- combine ----
    YD = np.concatenate([res.results[e]["yd"] for e in range(E)], axis=0)
    y = YD[gslot[:N]] + YD[gslot[N:]]
    y += np.concatenate([res.results[c]["ys"] for c in range(E)], axis=0)
    return y.reshape(B, T, C)
